# revision 33
# baseline (speedup 1.0000x reference)
"""MoE routing kernel for Trainium2 (8 NeuronCores, expert-parallel).

Problem: y[n] = x[n] @ W[index[n]].T + b[index[n]]
  x [16384, 1024] f32, index [16384] i32, W [8, 512, 1024] f32, b [8, 512] f32

Strategy (expert-parallel, dispatch on index during sharding):
  Core e owns expert e. The host groups rows by expert (the all-to-all
  dispatch), packs each core's rows into PE-friendly transposed tiles, and
  each core runs a dense [R,1024] @ [1024,512] matmul with its expert's
  weights. Results are scattered back to original row order on the host.

Device layout per core (one NEFF, SPMD on cores 0-7):
  xT  [RT, 128, 8, 128]  (row-tile, k%128, k-tile, r) — lhsT blocks; a
                         partition line (fixed k) is contiguous in DRAM
  wT  [8, 128, 512]      (k-tile, k, o)               — rhs blocks (moving)
  y   [RT, 128, 512]     (row-tile, r, o)
  For each row-tile: accumulate 8 matmuls over k-tiles into one PSUM bank,
  copy PSUM->SBUF on DVE, DMA out.
"""

from contextlib import ExitStack

import numpy as np

import concourse.bass as bass
import concourse.mybir as mybir
import concourse.tile as tile
from concourse import bacc
from concourse.bass_utils import run_bass_kernel_spmd

N_CORES = 8
D_IN = 1024
D_OUT = 512
KT = D_IN // 128  # 8 k-tiles

# matmul input dtypes (lhsT = x blocks, rhs = W blocks). float16 runs the
# PE at 1 column/cycle with fast weight load (fp32 is 4x slower, fp32r has
# no fast weight load) and halves the input DMA. Accuracy vs the fp32
# reference is ~3e-4 relative (10-bit mantissa; values here are well within
# fp16 range: |x| < ~6, |W| < ~0.06, accumulation in fp32 PSUM).
X_DT = mybir.dt.float16
W_DT = mybir.dt.float16

# Number of PE-warmup dummy matmuls (0 disables).
WARMUP_MMS = 0

# Skip the construction-time all-engine barrier (earlier first DMA).
SKIP_INIT_BARRIER = False


class _NoInitBarrierBacc(bacc.Bacc):
    """Bacc whose construction-time all-engine barrier is skipped.

    Bass.__init__ ends with an all-engine barrier whose only job is to order
    the const-pool memsets (which this kernel never reads) before the body.
    Skipping it lets each engine enter the body as soon as the runtime
    releases it, so the first DMAs issue ~4us earlier. All body dependencies
    are still fully managed by Tile's semaphores (initialized by the NEFF
    loader, not by engine code).
    """

    def all_engine_barrier(self, *, sem_only: bool = False):
        if not getattr(self, "_init_barrier_skipped", False):
            self._init_barrier_skipped = True
            return None
        return super().all_engine_barrier(sem_only=sem_only)


def build_nc(rt: int, x_dt=None, w_dt=None):
    """Build + compile the per-core Bass program for `rt` row-tiles."""
    x_dt = x_dt or X_DT
    w_dt = w_dt or W_DT
    nc = (_NoInitBarrierBacc if SKIP_INIT_BARRIER else bacc.Bacc)(
        "TRN2",
        target_bir_lowering=False,
        debug=False,
        enable_asserts=False,
        num_devices=N_CORES,
    )
    f32 = mybir.dt.float32
    xT = nc.dram_tensor("xT", [rt, 128, KT * 128], x_dt, kind="ExternalInput").ap()
    wT = nc.dram_tensor("wT", [KT, 128, D_OUT], w_dt, kind="ExternalInput").ap()
    y = nc.dram_tensor("y", [rt, 128, D_OUT], f32, kind="ExternalOutput").ap()

    with tile.TileContext(nc) as tc, ExitStack() as ctx:
        w_pool = ctx.enter_context(tc.tile_pool(name="w", bufs=1))
        x_pool = ctx.enter_context(tc.tile_pool(name="x", bufs=8))
        o_pool = ctx.enter_context(tc.tile_pool(name="o", bufs=8))
        p_pool = ctx.enter_context(tc.tile_pool(name="p", bufs=6, space="PSUM"))

        w_tiles = []
        for kt in range(KT):
            w_tiles.append(
                w_pool.tile([128, D_OUT], w_dt, tag=f"w{kt}", name=f"w{kt}")
            )

        # PE warmup: the HAM clock gate keeps the PE at 1.2 GHz until it has
        # been busy ~3.4us, and re-throttles after ~3.4us idle. With the
        # init barrier gone the DVE body starts ~6us, so a memset + dummy
        # matmul chain can warm the PE before the first real matmul (~16us)
        # and keep it warm (the remaining idle gap stays under the window).
        if WARMUP_MMS:
            warm_pool = ctx.enter_context(tc.tile_pool(name="warm", bufs=1))
            warm_sb = warm_pool.tile(
                [128, D_OUT], x_dt, tag="warm", name="warm_sb"
            )
            nc.vector.memset(warm_sb[:], 0.0)
            warm_ps = p_pool.tile(
                [128, D_OUT], f32, tag="warm_ps", name="warm_ps", bufs=1
            )
            for i in range(WARMUP_MMS):
                nc.tensor.matmul(
                    warm_ps[:], warm_sb[:, :128], warm_sb[:], start=True, stop=True
                )

        # Spread the matmul-gating transfers over all three DMA rings,
        # assigning W k-tiles round-robin so their arrival order matches the
        # first row-tile's accumulation order (~0.22us/tile merged rate):
        # scalar (ACT HWDGE) w0/w3/w6, gpsimd (SWDGE) w1/w4/w7, sync
        # (SP HWDGE) the x stream (row-tile 0 split for earlier first
        # k-slices) + w2/w5.
        x_tiles = {}
        x_t0 = x_pool.tile([128, KT * 128], x_dt, tag="x0", name="x0", bufs=1)
        nc.sync.dma_start(x_t0[:, : 2 * 128], xT[0, :, : 2 * 128])
        nc.sync.dma_start(x_t0[:, 2 * 128 :], xT[0, :, 2 * 128 :])
        x_tiles[0] = x_t0
        for kt in (0, 3, 6):
            nc.scalar.dma_start(w_tiles[kt][:], wT[kt])
        for kt in (1, 4, 7):
            nc.gpsimd.dma_start(w_tiles[kt][:], wT[kt])
        for kt in (2, 5):
            nc.sync.dma_start(w_tiles[kt][:], wT[kt])
        for r in range(1, rt):
            x_t = x_pool.tile([128, KT * 128], x_dt, tag="x", name=f"x{r}")
            nc.sync.dma_start(x_t[:], xT[r])
            x_tiles[r] = x_t

        for r in range(rt):
            x_t = x_tiles[r]
            psum = p_pool.tile([128, D_OUT], f32, tag="ps")
            for kt in range(KT):
                nc.tensor.matmul(
                    psum[:],
                    x_t[:, bass.ts(kt, 128)],
                    w_tiles[kt][:],
                    start=(kt == 0),
                    stop=(kt == KT - 1),
                )
            o_t = o_pool.tile([128, D_OUT], f32, tag="o")
            nc.vector.tensor_copy(o_t[:], psum[:])
            nc.scalar.dma_start(y[r], o_t[:])

    nc.compile()
    return nc


def make_in_maps(x, index, W, x_dt=None, w_dt=None):
    """Group rows by expert, pack per-core transposed tiles.

    Returns (in_maps, rows_per_expert, rt) where rows_per_expert[e] is the
    original row indices handled by core e.
    """
    import concourse.mybir as _mybir

    x_np = _mybir.dt.np(x_dt or X_DT)
    w_np = _mybir.dt.np(w_dt or W_DT)
    x = np.ascontiguousarray(x, dtype=np.float32)
    W = np.ascontiguousarray(W, dtype=np.float32)
    rows_per_expert = [np.nonzero(index == e)[0] for e in range(N_CORES)]
    max_rows = max(len(r) for r in rows_per_expert)
    rt = max((max_rows + 127) // 128, 1)
    r_pad = rt * 128

    in_maps = []
    for e in range(N_CORES):
        rows = rows_per_expert[e]
        xp = np.zeros((r_pad, D_IN), np.float32)
        xp[: len(rows)] = x[rows]
        # [R, D_IN] -> [RT, 128r, KT, 128k] -> [RT, 128k, KT, 128r]
        # so a partition line (fixed k) is KT*128 elements contiguous.
        xT = np.ascontiguousarray(
            xp.reshape(rt, 128, KT, 128).transpose(0, 3, 2, 1).reshape(rt, 128, -1),
            dtype=x_np,
        )
        wT = np.ascontiguousarray(W[e].T.reshape(KT, 128, D_OUT), dtype=w_np)
        in_maps.append({"xT": xT, "wT": wT})
    return in_maps, rows_per_expert, rt


def assemble_output(results, rows_per_expert, n_rows, index=None, b=None):
    y = np.zeros((n_rows, D_OUT), np.float32)
    for e, rows in enumerate(rows_per_expert):
        yc = results[e]["y"].reshape(-1, D_OUT)
        y[rows] = yc[: len(rows)]
    if b is not None and np.any(b):
        y += np.asarray(b, np.float32)[np.asarray(index)]
    return y


def kernel(x, index, W, b):
    x = np.asarray(x)
    index = np.asarray(index, np.int32)
    W = np.asarray(W)
    b = np.asarray(b)
    in_maps, rows_per_expert, rt = make_in_maps(x, index, W)
    nc = build_nc(rt)
    res = run_bass_kernel_spmd(nc, in_maps, core_ids=list(range(N_CORES)))
    return assemble_output(res.results, rows_per_expert, x.shape[0], index, b)
